# revision 15
# baseline (speedup 1.0000x reference)
"""Trainium2 Bass kernel for nn_DotProductAttention_11433202942822.

Math (per (b, h) pair, T=2048, D=64):
    S = Q @ K^T * (1/sqrt(64))            [T1, T2]
    attn = softmax(S, axis=T1)            <- softmax over the QUERY axis
    out = attn @ V                        [T1, D]

Key restructuring for TRN2:
  * Compute S^T = K @ Q^T with k2 on partitions and q on the free axis, so
    the softmax reduction (over q) is a free-axis reduction.
  * Fold the softmax normalization into V instead of the attention matrix:
        out^T[d, q] = sum_k2 (V[k2, d] / s[k2]) * E^T[k2, q]
  * exp() is split across engines per k2-tile (the ScalarE ACTIVATE was the
    86%-busy bottleneck of the single-engine version):
      - 'S' tiles: ScalarE activation(Exp) with fused accum_out sums.
      - 'D' tiles: VectorE Schraudolph bit-trick exp -- int16(a*s + b)
        reinterpreted as fp16 is exp(s*scale) to ~1.8% rms (mean-zero,
        washes out over the 2048-term contraction), then a gpsimd fp16
        add-tree (2048->1024->512) + VectorE tensor_reduce for the sums.
  * Per-head batched normalization: one reciprocal + one broadcast
    multiply for all 16 tiles (replaces 64 tiny DVE ops).
  * Matmuls in fp16, N=512 chunks, k2-tile pairs split across PE
    partition-half row-groups so mm1 runs tile-pairs concurrently.

Sharding: batch*heads = 32 pairs, 4 per core across 8 cores (head/data
parallel, no cross-core communication).
"""

import sys

import numpy as np

if "/opt/trn_rl_repo" not in sys.path:
    sys.path.insert(0, "/opt/trn_rl_repo")

import concourse.tile as tile  # noqa: E402
from concourse import bacc, mybir  # noqa: E402
from concourse.bass_utils import run_bass_kernel_spmd  # noqa: E402

P = 128
D = 64
SCALE = 1.0 / (D ** 0.5)
N_CORES = 8

F32 = mybir.dt.float32
F16 = mybir.dt.float16
I16 = mybir.dt.int16

LOG2E = 1.4426950408889634
A_IMM = float(SCALE * 1024.0 * LOG2E)   # score -> fp16-exponent domain
B_IMM = 15360.0 - 59.0                  # fp16 bias<<10, -59 zero-mean tweak

# Per-head engine assignment for the 16 k2-tiles (DVE Schraudolph tiles).
# Last head keeps its tail tiles on ScalarE so the final mm2s aren't gated
# on the slower gpsimd sum-tree.
DVE_SET = frozenset({1, 3, 5, 7, 9, 11, 13})
DVE_SET_LAST = frozenset({1, 3, 5, 7, 9, 11})
N_DEEP_LIMIT = 4
SPLIT_EVAC = True


def dve_tiles_for_head(bh: int, last: bool) -> frozenset:
    return DVE_SET_LAST if last else DVE_SET


def build_attention_nc(BH: int, T: int, debug: bool = False):
    """Per-core Bass module.

    Inputs (per core):
      qt  [BH, 2D, T]   fp16  Q^T duplicated on both partition halves
      kt  [BH, 2D, T/2] fp16  K^T, even k2-tiles on partitions 0-63, odd 64-127
      v   [BH, P, T/P, D] f32 V with k2 split (tile, partition)
    Output:
      out [BH, D, T]   f32   out^T (d-major)
    """
    assert T % 1024 == 0 and T % P == 0
    KT_TILES = T // P

    nc = bacc.Bacc("TRN2", target_bir_lowering=False, debug=debug)

    qt = nc.dram_tensor("qt", [BH, 2 * D, T], F16, kind="ExternalInput").ap()
    kt = nc.dram_tensor("kt", [BH, 2 * D, T // 2], F16, kind="ExternalInput").ap()
    v = nc.dram_tensor("v", [BH, P, T // P, D], F32, kind="ExternalInput").ap()
    out = nc.dram_tensor("out", [BH, D, T], F32, kind="ExternalOutput").ap()

    with tile.TileContext(nc) as tc:
        with (
            tc.tile_pool(name="ins", bufs=1) as ins_pool,
            tc.tile_pool(name="et", bufs=26) as et_pool,
            tc.tile_pool(name="ph", bufs=3) as ph_pool,
            tc.tile_pool(name="small", bufs=6) as small_pool,
            tc.tile_pool(name="osb", bufs=2) as osb_pool,
            tc.tile_pool(name="spsum", bufs=3, space="PSUM") as s_pool,
            tc.tile_pool(name="opsum", bufs=1, space="PSUM") as o_pool,
        ):
            qt_sb = ins_pool.tile([2 * D, BH, T], F16, tag="qt_sb")
            kt_sb = ins_pool.tile([2 * D, BH, T // 2], F16, tag="kt_sb")
            v_sb = ins_pool.tile([P, BH, KT_TILES, D], F32, tag="v_sb")
            # Warm the ACT exp table during input DMA.
            warm = small_pool.tile([P, 1], F32, tag="warm")
            nc.vector.memset(warm[:], 0.0)
            nc.scalar.activation(
                warm[:], warm[:], mybir.ActivationFunctionType.Exp
            )

            for bh in range(BH):
                nc.sync.dma_start(qt_sb[:, bh, :], qt[bh])
                nc.sync.dma_start(kt_sb[:, bh, :], kt[bh])
                nc.sync.dma_start(v_sb[:, bh], v[bh])

            def emit_mm2(state, t):
                out_ps, ets = state["out_ps"], state["ets"]
                vp = state["vp"]
                et_f16 = ets[t].bitcast(F16)
                for c in range(0, T, 512):
                    half = c // (T // 2)
                    qh = c % (T // 2)
                    nc.tensor.matmul(
                        out_ps[half * D:(half + 1) * D, qh:qh + 512],
                        lhsT=vp[:, t, :],
                        rhs=et_f16[:, c:c + 512],
                        start=(t == 0),
                        stop=(t == KT_TILES - 1),
                        skip_group_check=True,
                    )

            def evacuate(bh, out_ps):
                if not SPLIT_EVAC:
                    osb = osb_pool.tile([2 * D, T // 2], F32, tag="osbf")
                    nc.vector.tensor_copy(osb[:], out_ps[:])
                    nc.sync.dma_start(out[bh][:, 0:T // 2], osb[0:D])
                    nc.sync.dma_start(out[bh][:, T // 2:T], osb[D:2 * D])
                    return
                # Two half-copies so the first DMA overlaps the second copy.
                for h in range(2):
                    osb = osb_pool.tile([2 * D, T // 4], F32, tag="osb")
                    sl = slice(h * (T // 4), (h + 1) * (T // 4))
                    nc.vector.tensor_copy(osb[:], out_ps[:, sl])
                    nc.sync.dma_start(out[bh][:, h * (T // 4):(h + 1) * (T // 4)],
                                      osb[0:D])
                    nc.sync.dma_start(
                        out[bh][:, T // 2 + h * (T // 4):T // 2 + (h + 1) * (T // 4)],
                        osb[D:2 * D])

            def pop_mm2(pending):
                state, t_ = pending.pop(0)
                emit_mm2(state, t_)
                if t_ == KT_TILES - 1:
                    evacuate(state["bh"], state["out_ps"])

            pending_mm2 = []
            for bh in range(BH):
                zc0 = small_pool.tile([P, KT_TILES], F32, tag="zc0")
                zc1 = small_pool.tile([P, KT_TILES], F32, tag="zc1")
                nc.vector.memset(zc1[:], 0.0)
                ets = {}
                vp = small_pool.tile([P, KT_TILES, D], F16, tag="vp")
                out_ps = o_pool.tile([2 * D, T // 2], F32, tag="out_ps")
                head_state = {"bh": bh, "out_ps": out_ps, "ets": ets, "vp": vp}

                def emit_norm(bh_, t0, n, zc0=zc0, zc1=zc1, vp=vp):
                    ztot = small_pool.tile([P, n], F32, tag="ztot")
                    nc.vector.tensor_add(
                        ztot[:], zc0[:, t0:t0 + n], zc1[:, t0:t0 + n]
                    )
                    rec = small_pool.tile([P, n], F32, tag="rec")
                    nc.vector.reciprocal(rec[:], ztot[:])
                    nc.vector.scalar_tensor_tensor(
                        vp[:, t0:t0 + n, :], v_sb[:, bh_, t0:t0 + n, :], 1.0,
                        rec[:, :, None].broadcast_to([P, n, D]),
                        mybir.AluOpType.mult, mybir.AluOpType.mult,
                    )
                dve_tiles = dve_tiles_for_head(bh, bh == BH - 1)
                n_deep = 0  # 3-level gpsimd trees used so far this head
                for j in range(KT_TILES // 2):
                    tA, tB = 2 * j, 2 * j + 1
                    ets[tA] = et_pool.tile([P, T], I16, tag="et", name="etA")
                    ets[tB] = et_pool.tile([P, T], I16, tag="et", name="etB")
                    lhsA = kt_sb[0:D, bh, j * P:(j + 1) * P]
                    lhsB = kt_sb[D:2 * D, bh, j * P:(j + 1) * P]
                    for q0 in range(0, T, 1024):
                        spA = s_pool.tile([P, 1024], F32, tag="sp", name="spA")
                        spB = s_pool.tile([P, 1024], F32, tag="sp", name="spB")
                        for c in range(0, 1024, 512):
                            nc.tensor.matmul(
                                spA[:, c:c + 512],
                                lhsT=lhsA,
                                rhs=qt_sb[0:D, bh, q0 + c:q0 + c + 512],
                                start=True,
                                stop=True,
                            )
                            nc.tensor.matmul(
                                spB[:, c:c + 512],
                                lhsT=lhsB,
                                rhs=qt_sb[D:2 * D, bh, q0 + c:q0 + c + 512],
                                start=True,
                                stop=True,
                            )
                        for t, sp in ((tA, spA), (tB, spB)):
                            if t in dve_tiles:
                                nc.vector.tensor_scalar(
                                    ets[t][:, q0:q0 + 1024], sp[:],
                                    A_IMM, B_IMM,
                                    mybir.AluOpType.mult,
                                    mybir.AluOpType.add,
                                )
                            else:
                                zc = zc0 if q0 == 0 else zc1
                                nc.scalar.activation(
                                    ets[t].bitcast(F16)[:, q0:q0 + 1024],
                                    sp[:],
                                    mybir.ActivationFunctionType.Exp,
                                    scale=SCALE,
                                    accum_out=zc[:, t:t + 1],
                                )
                    # Sum path for DVE tiles: gpsimd fp16 add-tree (depth
                    # balanced against gpsimd saturation), small DVE reduce.
                    for t in (tA, tB):
                        if t not in dve_tiles:
                            continue
                        etf = ets[t].bitcast(F16)
                        h1 = ph_pool.tile([P, T // 2], F16, tag="h1")
                        h2 = ph_pool.tile([P, T // 4], F16, tag="h2")
                        nc.gpsimd.tensor_add(
                            h1[:], etf[:, 0:T // 2], etf[:, T // 2:T]
                        )
                        nc.gpsimd.tensor_add(
                            h2[:], h1[:, 0:T // 4], h1[:, T // 4:T // 2]
                        )
                        if n_deep < N_DEEP_LIMIT:
                            n_deep += 1
                            h3 = ph_pool.tile([P, T // 8], F16, tag="h3")
                            nc.gpsimd.tensor_add(
                                h3[:], h2[:, 0:T // 8], h2[:, T // 8:T // 4]
                            )
                            red_src = h3
                        else:
                            red_src = h2
                        nc.vector.tensor_reduce(
                            zc0[:, t:t + 1], red_src[:],
                            mybir.AxisListType.X, mybir.AluOpType.add,
                        )
                    # Last head: per-pair Z/vp so its mm2s pipeline instead
                    # of piling into a dead tail after all exps finish.
                    if bh == BH - 1:
                        emit_norm(bh, 2 * j, 2)
                        pending_mm2.append((head_state, 2 * j))
                        pending_mm2.append((head_state, 2 * j + 1))
                        # Lag pops two pairs behind the norm emission so the
                        # PE never head-of-line blocks on an unready vp.
                        while len(pending_mm2) > 4:
                            pop_mm2(pending_mm2)
                    else:
                        # Drain the previous head's mm2s (vp long ready),
                        # at most 3 per pair so PE bursts stay small and
                        # ScalarE keeps getting fresh mm1 scores.
                        pops = 0
                        while len(pending_mm2) > 4 and pops < 3:
                            pop_mm2(pending_mm2)
                            pops += 1

                if bh < BH - 1:
                    # Batched per-head normalization.
                    emit_norm(bh, 0, KT_TILES)
                    for t in range(KT_TILES):
                        pending_mm2.append((head_state, t))
            while pending_mm2:
                pop_mm2(pending_mm2)

    nc.compile()
    return nc


_NC_CACHE: dict = {}

TRACE = False
LAST_RESULTS = None


def _get_nc(BH: int, T: int):
    key = (BH, T)
    if key not in _NC_CACHE:
        _NC_CACHE[key] = build_attention_nc(BH, T)
    return _NC_CACHE[key]


def _reference_numpy(Q, K, V, padding_mask, isCausal):
    """Fallback exactly mirroring reference.py (never hit for spec inputs)."""
    Q = Q.astype(np.float64)
    K = K.astype(np.float64)
    V = V.astype(np.float64)
    scores = np.einsum("bhqd,bhkd->bhqk", Q, K) * SCALE
    T1 = scores.shape[2]
    mask = padding_mask[:, None, :, :].astype(np.float64)
    if isCausal:
        mask = mask * np.tril(np.ones((T1, T1)))
    scores = np.where(mask == 0, -np.inf, scores)
    m = np.max(scores, axis=2, keepdims=True)
    e = np.exp(scores - m)
    attn = e / np.sum(e, axis=2, keepdims=True)
    return np.einsum("bhqk,bhkd->bhqd", attn, V).astype(np.float32)


def kernel(Q, K, V, padding_mask, isCausal, **_unused):
    Q = np.asarray(Q, dtype=np.float32)
    K = np.asarray(K, dtype=np.float32)
    V = np.asarray(V, dtype=np.float32)
    padding_mask = np.asarray(padding_mask)
    causal = int(np.asarray(isCausal))

    B, H, T, Dd = Q.shape
    assert Dd == D
    if causal != 0 or padding_mask.min() != 1.0 or padding_mask.max() != 1.0:
        return _reference_numpy(Q, K, V, padding_mask, causal)

    BHT = B * H
    assert BHT % N_CORES == 0
    BH = BHT // N_CORES

    nc = _get_nc(BH, T)

    Qf = Q.reshape(BHT, T, D)
    Kf = K.reshape(BHT, T, D)
    Vf = V.reshape(BHT, T, D)

    QT = Qf.transpose(0, 2, 1).astype(np.float16)
    qt_all = np.ascontiguousarray(np.concatenate([QT, QT], axis=1))
    KT = Kf.transpose(0, 2, 1).astype(np.float16)
    KT4 = KT.reshape(BHT, D, T // 128, 128)
    kt_all = np.ascontiguousarray(
        np.concatenate(
            [
                KT4[:, :, 0::2, :].reshape(BHT, D, T // 2),
                KT4[:, :, 1::2, :].reshape(BHT, D, T // 2),
            ],
            axis=1,
        )
    )
    v_all = np.ascontiguousarray(
        Vf.reshape(BHT, T // P, P, D).transpose(0, 2, 1, 3)
    )

    in_maps = []
    for c in range(N_CORES):
        sl = slice(c * BH, (c + 1) * BH)
        in_maps.append(
            {
                "qt": np.ascontiguousarray(qt_all[sl]),
                "kt": np.ascontiguousarray(kt_all[sl]),
                "v": np.ascontiguousarray(v_all[sl]),
            }
        )

    res = None
    last_err = None
    for attempt in range(3):
        try:
            res = run_bass_kernel_spmd(
                nc, in_maps, core_ids=list(range(N_CORES)), trace=TRACE
            )
            break
        except Exception as e:
            last_err = e
            import time as _time

            _time.sleep(2.0)
    if res is None:
        raise last_err
    global LAST_RESULTS
    LAST_RESULTS = res

    outs = [res.results[c]["out"] for c in range(N_CORES)]
    out_all = np.concatenate(outs, axis=0)
    out = out_all.transpose(0, 2, 1).reshape(B, H, T, D)
    return np.ascontiguousarray(out).astype(np.float32)


# revision 17
# speedup vs baseline: 1.0864x; 1.0864x over previous
"""Trainium2 Bass kernel for nn_DotProductAttention_11433202942822.

Math (per (b, h) pair, T=2048, D=64):
    S = Q @ K^T * (1/sqrt(64))            [T1, T2]
    attn = softmax(S, axis=T1)            <- softmax over the QUERY axis
    out = attn @ V                        [T1, D]

Key restructuring for TRN2:
  * Compute S^T = K @ Q^T with k2 on partitions and q on the free axis, so
    the softmax reduction (over q) is a free-axis reduction.
  * Fold the softmax normalization into V instead of the attention matrix:
        out^T[d, q] = sum_k2 (V[k2, d] / s[k2]) * E^T[k2, q]
  * exp() is split across engines per k2-tile (the ScalarE ACTIVATE was the
    86%-busy bottleneck of the single-engine version):
      - 'S' tiles: ScalarE activation(Exp) with fused accum_out sums.
      - 'D' tiles: VectorE Schraudolph bit-trick exp -- int16(a*s + b)
        reinterpreted as fp16 is exp(s*scale) to ~1.8% rms (mean-zero,
        washes out over the 2048-term contraction), then a gpsimd fp16
        add-tree (2048->1024->512) + VectorE tensor_reduce for the sums.
  * Per-head batched normalization: one reciprocal + one broadcast
    multiply for all 16 tiles (replaces 64 tiny DVE ops).
  * Matmuls in fp16, N=512 chunks, k2-tile pairs split across PE
    partition-half row-groups so mm1 runs tile-pairs concurrently.

Sharding: batch*heads = 32 pairs, 4 per core across 8 cores (head/data
parallel, no cross-core communication).
"""

import sys

import numpy as np

if "/opt/trn_rl_repo" not in sys.path:
    sys.path.insert(0, "/opt/trn_rl_repo")

import concourse.tile as tile  # noqa: E402
from concourse import bacc, mybir  # noqa: E402
from concourse.bass_utils import run_bass_kernel_spmd  # noqa: E402

P = 128
D = 64
SCALE = 1.0 / (D ** 0.5)
N_CORES = 8

F32 = mybir.dt.float32
F16 = mybir.dt.float16
I16 = mybir.dt.int16

LOG2E = 1.4426950408889634
A_IMM = float(SCALE * 1024.0 * LOG2E)   # score -> fp16-exponent domain
B_IMM = 15360.0 - 59.0                  # fp16 bias<<10, -59 zero-mean tweak

# Per-head engine assignment for the 16 k2-tiles (DVE Schraudolph tiles).
# Last head keeps its tail tiles on ScalarE so the final mm2s aren't gated
# on the slower gpsimd sum-tree.
DVE_SET = frozenset({1, 3, 6, 9, 11, 14})
DVE_SET_LAST = frozenset({1, 3, 6, 9, 11})
N_DEEP_LIMIT = 0
SPLIT_EVAC = True


def dve_tiles_for_head(bh: int, last: bool) -> frozenset:
    return DVE_SET_LAST if last else DVE_SET


def build_attention_nc(BH: int, T: int, debug: bool = False):
    """Per-core Bass module.

    Inputs (per core):
      qt  [BH, 2D, T]   fp16  Q^T duplicated on both partition halves
      kt  [BH, 2D, T/2] fp16  K^T, even k2-tiles on partitions 0-63, odd 64-127
      v   [BH, P, T/P, D] f32 V with k2 split (tile, partition)
    Output:
      out [BH, D, T]   f32   out^T (d-major)
    """
    assert T % 1024 == 0 and T % P == 0
    KT_TILES = T // P

    nc = bacc.Bacc("TRN2", target_bir_lowering=False, debug=debug)

    qt = nc.dram_tensor("qt", [BH, 2 * D, T], F16, kind="ExternalInput").ap()
    kt = nc.dram_tensor("kt", [BH, 2 * D, T // 2], F16, kind="ExternalInput").ap()
    v = nc.dram_tensor("v", [BH, P, T // P, D], F32, kind="ExternalInput").ap()
    out = nc.dram_tensor("out", [BH, D, T], F32, kind="ExternalOutput").ap()

    with tile.TileContext(nc) as tc:
        with (
            tc.tile_pool(name="ins", bufs=1) as ins_pool,
            tc.tile_pool(name="et", bufs=26) as et_pool,
            tc.tile_pool(name="ph", bufs=3) as ph_pool,
            tc.tile_pool(name="small", bufs=6) as small_pool,
            tc.tile_pool(name="osb", bufs=2) as osb_pool,
            tc.tile_pool(name="spsum", bufs=3, space="PSUM") as s_pool,
            tc.tile_pool(name="opsum", bufs=1, space="PSUM") as o_pool,
        ):
            qt_sb = ins_pool.tile([2 * D, BH, T], F16, tag="qt_sb")
            kt_sb = ins_pool.tile([2 * D, BH, T // 2], F16, tag="kt_sb")
            v_sb = ins_pool.tile([P, BH, KT_TILES, D], F32, tag="v_sb")
            # Warm the ACT exp table during input DMA.
            warm = small_pool.tile([P, 1], F32, tag="warm")
            nc.vector.memset(warm[:], 0.0)
            nc.scalar.activation(
                warm[:], warm[:], mybir.ActivationFunctionType.Exp
            )

            for bh in range(BH):
                nc.sync.dma_start(qt_sb[:, bh, :], qt[bh])
                nc.sync.dma_start(kt_sb[:, bh, :], kt[bh])
                nc.sync.dma_start(v_sb[:, bh], v[bh])

            def emit_mm2(state, t):
                out_ps, ets = state["out_ps"], state["ets"]
                vp = state["vp"]
                et_f16 = ets[t].bitcast(F16)
                for c in range(0, T, 512):
                    half = c // (T // 2)
                    qh = c % (T // 2)
                    nc.tensor.matmul(
                        out_ps[half * D:(half + 1) * D, qh:qh + 512],
                        lhsT=vp[:, t, :],
                        rhs=et_f16[:, c:c + 512],
                        start=(t == 0),
                        stop=(t == KT_TILES - 1),
                        skip_group_check=True,
                    )

            def evacuate(bh, out_ps):
                if not SPLIT_EVAC:
                    osb = osb_pool.tile([2 * D, T // 2], F32, tag="osbf")
                    nc.vector.tensor_copy(osb[:], out_ps[:])
                    nc.sync.dma_start(out[bh][:, 0:T // 2], osb[0:D])
                    nc.sync.dma_start(out[bh][:, T // 2:T], osb[D:2 * D])
                    return
                # Two half-copies so the first DMA overlaps the second copy.
                for h in range(2):
                    osb = osb_pool.tile([2 * D, T // 4], F32, tag="osb")
                    sl = slice(h * (T // 4), (h + 1) * (T // 4))
                    nc.vector.tensor_copy(osb[:], out_ps[:, sl])
                    nc.sync.dma_start(out[bh][:, h * (T // 4):(h + 1) * (T // 4)],
                                      osb[0:D])
                    nc.sync.dma_start(
                        out[bh][:, T // 2 + h * (T // 4):T // 2 + (h + 1) * (T // 4)],
                        osb[D:2 * D])

            def pop_mm2(pending):
                state, t_ = pending.pop(0)
                emit_mm2(state, t_)
                if t_ == KT_TILES - 1:
                    evacuate(state["bh"], state["out_ps"])

            pending_mm2 = []
            for bh in range(BH):
                zc0 = small_pool.tile([P, KT_TILES], F32, tag="zc0")
                zc1 = small_pool.tile([P, KT_TILES], F32, tag="zc1")
                nc.vector.memset(zc1[:], 0.0)
                ets = {}
                vp = small_pool.tile([P, KT_TILES, D], F16, tag="vp")
                out_ps = o_pool.tile([2 * D, T // 2], F32, tag="out_ps")
                head_state = {"bh": bh, "out_ps": out_ps, "ets": ets, "vp": vp}

                def emit_norm(bh_, t0, n, zc0=zc0, zc1=zc1, vp=vp):
                    ztot = small_pool.tile([P, n], F32, tag="ztot")
                    nc.vector.tensor_add(
                        ztot[:], zc0[:, t0:t0 + n], zc1[:, t0:t0 + n]
                    )
                    rec = small_pool.tile([P, n], F32, tag="rec")
                    nc.vector.reciprocal(rec[:], ztot[:])
                    nc.vector.scalar_tensor_tensor(
                        vp[:, t0:t0 + n, :], v_sb[:, bh_, t0:t0 + n, :], 1.0,
                        rec[:, :, None].broadcast_to([P, n, D]),
                        mybir.AluOpType.mult, mybir.AluOpType.mult,
                    )
                dve_tiles = dve_tiles_for_head(bh, bh == BH - 1)
                n_deep = 0  # 3-level gpsimd trees used so far this head
                for j in range(KT_TILES // 2):
                    tA, tB = 2 * j, 2 * j + 1
                    ets[tA] = et_pool.tile([P, T], I16, tag="et", name="etA")
                    ets[tB] = et_pool.tile([P, T], I16, tag="et", name="etB")
                    lhsA = kt_sb[0:D, bh, j * P:(j + 1) * P]
                    lhsB = kt_sb[D:2 * D, bh, j * P:(j + 1) * P]
                    for q0 in range(0, T, 1024):
                        spA = s_pool.tile([P, 1024], F32, tag="sp", name="spA")
                        spB = s_pool.tile([P, 1024], F32, tag="sp", name="spB")
                        for c in range(0, 1024, 512):
                            nc.tensor.matmul(
                                spA[:, c:c + 512],
                                lhsT=lhsA,
                                rhs=qt_sb[0:D, bh, q0 + c:q0 + c + 512],
                                start=True,
                                stop=True,
                            )
                            nc.tensor.matmul(
                                spB[:, c:c + 512],
                                lhsT=lhsB,
                                rhs=qt_sb[D:2 * D, bh, q0 + c:q0 + c + 512],
                                start=True,
                                stop=True,
                            )
                        for t, sp in ((tA, spA), (tB, spB)):
                            if t in dve_tiles:
                                nc.vector.tensor_scalar(
                                    ets[t][:, q0:q0 + 1024], sp[:],
                                    A_IMM, B_IMM,
                                    mybir.AluOpType.mult,
                                    mybir.AluOpType.add,
                                )
                            else:
                                zc = zc0 if q0 == 0 else zc1
                                nc.scalar.activation(
                                    ets[t].bitcast(F16)[:, q0:q0 + 1024],
                                    sp[:],
                                    mybir.ActivationFunctionType.Exp,
                                    scale=SCALE,
                                    accum_out=zc[:, t:t + 1],
                                )
                    # Sum path for DVE tiles: gpsimd fp16 add-tree (depth
                    # balanced against gpsimd saturation), small DVE reduce.
                    for t in (tA, tB):
                        if t not in dve_tiles:
                            continue
                        etf = ets[t].bitcast(F16)
                        h1 = ph_pool.tile([P, T // 2], F16, tag="h1")
                        h2 = ph_pool.tile([P, T // 4], F16, tag="h2")
                        nc.gpsimd.tensor_add(
                            h1[:], etf[:, 0:T // 2], etf[:, T // 2:T]
                        )
                        nc.gpsimd.tensor_add(
                            h2[:], h1[:, 0:T // 4], h1[:, T // 4:T // 2]
                        )
                        if n_deep < N_DEEP_LIMIT:
                            n_deep += 1
                            h3 = ph_pool.tile([P, T // 8], F16, tag="h3")
                            nc.gpsimd.tensor_add(
                                h3[:], h2[:, 0:T // 8], h2[:, T // 8:T // 4]
                            )
                            red_src = h3
                        else:
                            red_src = h2
                        nc.vector.tensor_reduce(
                            zc0[:, t:t + 1], red_src[:],
                            mybir.AxisListType.X, mybir.AluOpType.add,
                        )
                    # Last head: per-pair Z/vp so its mm2s pipeline instead
                    # of piling into a dead tail after all exps finish.
                    if bh == BH - 1:
                        emit_norm(bh, 2 * j, 2)
                        pending_mm2.append((head_state, 2 * j))
                        pending_mm2.append((head_state, 2 * j + 1))
                        # Lag pops two pairs behind the norm emission so the
                        # PE never head-of-line blocks on an unready vp.
                        while len(pending_mm2) > 4:
                            pop_mm2(pending_mm2)
                    else:
                        # Drain the previous head's mm2s (vp long ready).
                        while len(pending_mm2) > 4:
                            pop_mm2(pending_mm2)

                if bh < BH - 1:
                    # Batched per-head normalization.
                    emit_norm(bh, 0, KT_TILES)
                    for t in range(KT_TILES):
                        pending_mm2.append((head_state, t))
            while pending_mm2:
                pop_mm2(pending_mm2)

    nc.compile()
    return nc


_NC_CACHE: dict = {}

TRACE = False
LAST_RESULTS = None


def _get_nc(BH: int, T: int):
    key = (BH, T)
    if key not in _NC_CACHE:
        _NC_CACHE[key] = build_attention_nc(BH, T)
    return _NC_CACHE[key]


def _reference_numpy(Q, K, V, padding_mask, isCausal):
    """Fallback exactly mirroring reference.py (never hit for spec inputs)."""
    Q = Q.astype(np.float64)
    K = K.astype(np.float64)
    V = V.astype(np.float64)
    scores = np.einsum("bhqd,bhkd->bhqk", Q, K) * SCALE
    T1 = scores.shape[2]
    mask = padding_mask[:, None, :, :].astype(np.float64)
    if isCausal:
        mask = mask * np.tril(np.ones((T1, T1)))
    scores = np.where(mask == 0, -np.inf, scores)
    m = np.max(scores, axis=2, keepdims=True)
    e = np.exp(scores - m)
    attn = e / np.sum(e, axis=2, keepdims=True)
    return np.einsum("bhqk,bhkd->bhqd", attn, V).astype(np.float32)


def kernel(Q, K, V, padding_mask, isCausal, **_unused):
    Q = np.asarray(Q, dtype=np.float32)
    K = np.asarray(K, dtype=np.float32)
    V = np.asarray(V, dtype=np.float32)
    padding_mask = np.asarray(padding_mask)
    causal = int(np.asarray(isCausal))

    B, H, T, Dd = Q.shape
    assert Dd == D
    if causal != 0 or padding_mask.min() != 1.0 or padding_mask.max() != 1.0:
        return _reference_numpy(Q, K, V, padding_mask, causal)

    BHT = B * H
    assert BHT % N_CORES == 0
    BH = BHT // N_CORES

    nc = _get_nc(BH, T)

    Qf = Q.reshape(BHT, T, D)
    Kf = K.reshape(BHT, T, D)
    Vf = V.reshape(BHT, T, D)

    QT = Qf.transpose(0, 2, 1).astype(np.float16)
    qt_all = np.ascontiguousarray(np.concatenate([QT, QT], axis=1))
    KT = Kf.transpose(0, 2, 1).astype(np.float16)
    KT4 = KT.reshape(BHT, D, T // 128, 128)
    kt_all = np.ascontiguousarray(
        np.concatenate(
            [
                KT4[:, :, 0::2, :].reshape(BHT, D, T // 2),
                KT4[:, :, 1::2, :].reshape(BHT, D, T // 2),
            ],
            axis=1,
        )
    )
    v_all = np.ascontiguousarray(
        Vf.reshape(BHT, T // P, P, D).transpose(0, 2, 1, 3)
    )

    in_maps = []
    for c in range(N_CORES):
        sl = slice(c * BH, (c + 1) * BH)
        in_maps.append(
            {
                "qt": np.ascontiguousarray(qt_all[sl]),
                "kt": np.ascontiguousarray(kt_all[sl]),
                "v": np.ascontiguousarray(v_all[sl]),
            }
        )

    res = None
    last_err = None
    for attempt in range(3):
        try:
            res = run_bass_kernel_spmd(
                nc, in_maps, core_ids=list(range(N_CORES)), trace=TRACE
            )
            break
        except Exception as e:
            last_err = e
            import time as _time

            _time.sleep(2.0)
    if res is None:
        raise last_err
    global LAST_RESULTS
    LAST_RESULTS = res

    outs = [res.results[c]["out"] for c in range(N_CORES)]
    out_all = np.concatenate(outs, axis=0)
    out = out_all.transpose(0, 2, 1).reshape(B, H, T, D)
    return np.ascontiguousarray(out).astype(np.float32)


# revision 19
# speedup vs baseline: 1.1013x; 1.0136x over previous
"""Trainium2 Bass kernel for nn_DotProductAttention_11433202942822.

Math (per (b, h) pair, T=2048, D=64):
    S = Q @ K^T * (1/sqrt(64))            [T1, T2]
    attn = softmax(S, axis=T1)            <- softmax over the QUERY axis
    out = attn @ V                        [T1, D]

Key restructuring for TRN2:
  * Compute S^T = K @ Q^T with k2 on partitions and q on the free axis, so
    the softmax reduction (over q) is a free-axis reduction.
  * Fold the softmax normalization into V instead of the attention matrix:
        out^T[d, q] = sum_k2 (V[k2, d] / s[k2]) * E^T[k2, q]
  * exp() is split across engines per k2-tile (the ScalarE ACTIVATE was the
    86%-busy bottleneck of the single-engine version):
      - 'S' tiles: ScalarE activation(Exp) with fused accum_out sums.
      - 'D' tiles: VectorE Schraudolph bit-trick exp -- int16(a*s + b)
        reinterpreted as fp16 is exp(s*scale) to ~1.8% rms (mean-zero,
        washes out over the 2048-term contraction), then a gpsimd fp16
        add-tree (2048->1024->512) + VectorE tensor_reduce for the sums.
  * Per-head batched normalization: one reciprocal + one broadcast
    multiply for all 16 tiles (replaces 64 tiny DVE ops).
  * Matmuls in fp16, N=512 chunks, k2-tile pairs split across PE
    partition-half row-groups so mm1 runs tile-pairs concurrently.

Sharding: batch*heads = 32 pairs, 4 per core across 8 cores (head/data
parallel, no cross-core communication).
"""

import sys

import numpy as np

if "/opt/trn_rl_repo" not in sys.path:
    sys.path.insert(0, "/opt/trn_rl_repo")

import concourse.tile as tile  # noqa: E402
from concourse import bacc, mybir  # noqa: E402
from concourse.bass_utils import run_bass_kernel_spmd  # noqa: E402

P = 128
D = 64
SCALE = 1.0 / (D ** 0.5)
N_CORES = 8

F32 = mybir.dt.float32
F16 = mybir.dt.float16
I16 = mybir.dt.int16

LOG2E = 1.4426950408889634
A_IMM = float(SCALE * 1024.0 * LOG2E)   # score -> fp16-exponent domain
B_IMM = 15360.0 - 59.0                  # fp16 bias<<10, -59 zero-mean tweak

# Per-head engine assignment for the 16 k2-tiles (DVE Schraudolph tiles).
# Last head keeps its tail tiles on ScalarE so the final mm2s aren't gated
# on the slower gpsimd sum-tree.
DVE_SET = frozenset({1, 3, 6, 9, 11, 14})
DVE_SET_LAST = frozenset({1, 3, 6, 9, 11})
N_DEEP_LIMIT = 0
SPLIT_EVAC = True


def dve_tiles_for_head(bh: int, last: bool) -> frozenset:
    return DVE_SET_LAST if last else DVE_SET


def build_attention_nc(BH: int, T: int, debug: bool = False):
    """Per-core Bass module.

    Inputs (per core):
      qt  [BH, 2D, T]   fp16  Q^T duplicated on both partition halves
      kt  [BH, 2D, T/2] fp16  K^T, even k2-tiles on partitions 0-63, odd 64-127
      v   [BH, P, T/P, D] f32 V with k2 split (tile, partition)
    Output:
      out [BH, D, T]   f32   out^T (d-major)
    """
    assert T % 1024 == 0 and T % P == 0
    KT_TILES = T // P

    nc = bacc.Bacc("TRN2", target_bir_lowering=False, debug=debug)

    qt = nc.dram_tensor("qt", [BH, 2 * D, T], F16, kind="ExternalInput").ap()
    kt = nc.dram_tensor("kt", [BH, 2 * D, T // 2], F16, kind="ExternalInput").ap()
    v = nc.dram_tensor("v", [BH, P, T // P, D], F32, kind="ExternalInput").ap()
    out = nc.dram_tensor("out", [BH, D, T], F32, kind="ExternalOutput").ap()

    with tile.TileContext(nc) as tc:
        with (
            tc.tile_pool(name="ins", bufs=1) as ins_pool,
            tc.tile_pool(name="et", bufs=26) as et_pool,
            tc.tile_pool(name="ph", bufs=3) as ph_pool,
            tc.tile_pool(name="small", bufs=6) as small_pool,
            tc.tile_pool(name="osb", bufs=2) as osb_pool,
            tc.tile_pool(name="spsum", bufs=3, space="PSUM") as s_pool,
            tc.tile_pool(name="opsum", bufs=1, space="PSUM") as o_pool,
        ):
            qt_sb = ins_pool.tile([2 * D, BH, T], F16, tag="qt_sb")
            kt_sb = ins_pool.tile([2 * D, BH, T // 2], F16, tag="kt_sb")
            v_sb = ins_pool.tile([P, BH, KT_TILES, D], F32, tag="v_sb")
            # Warm the ACT exp table during input DMA.
            warm = small_pool.tile([P, 1], F32, tag="warm")
            nc.vector.memset(warm[:], 0.0)
            nc.scalar.activation(
                warm[:], warm[:], mybir.ActivationFunctionType.Exp
            )

            for bh in range(BH):
                nc.sync.dma_start(qt_sb[:, bh, :], qt[bh])
                nc.sync.dma_start(kt_sb[:, bh, :], kt[bh])
                nc.sync.dma_start(v_sb[:, bh], v[bh])

            def emit_mm2(state, t):
                out_ps, ets = state["out_ps"], state["ets"]
                vp = state["vp"]
                et_f16 = ets[t].bitcast(F16)
                for c in range(0, T, 512):
                    half = c // (T // 2)
                    qh = c % (T // 2)
                    nc.tensor.matmul(
                        out_ps[half * D:(half + 1) * D, qh:qh + 512],
                        lhsT=vp[:, t, :],
                        rhs=et_f16[:, c:c + 512],
                        start=(t == 0),
                        stop=(t == KT_TILES - 1),
                        skip_group_check=True,
                    )

            def evacuate(bh, out_ps):
                if not SPLIT_EVAC:
                    osb = osb_pool.tile([2 * D, T // 2], F32, tag="osbf")
                    nc.vector.tensor_copy(osb[:], out_ps[:])
                    nc.sync.dma_start(out[bh][:, 0:T // 2], osb[0:D])
                    nc.sync.dma_start(out[bh][:, T // 2:T], osb[D:2 * D])
                    return
                # Two half-copies so the first DMA overlaps the second copy.
                # The last head's copies run on ScalarE (idle by then; DVE
                # still owns the final norm chain).
                for h in range(2):
                    osb = osb_pool.tile([2 * D, T // 4], F32, tag="osb")
                    sl = slice(h * (T // 4), (h + 1) * (T // 4))
                    if bh == BH - 1:
                        nc.scalar.copy(osb[:], out_ps[:, sl])
                    else:
                        nc.vector.tensor_copy(osb[:], out_ps[:, sl])
                    nc.sync.dma_start(out[bh][:, h * (T // 4):(h + 1) * (T // 4)],
                                      osb[0:D])
                    nc.sync.dma_start(
                        out[bh][:, T // 2 + h * (T // 4):T // 2 + (h + 1) * (T // 4)],
                        osb[D:2 * D])

            def pop_mm2(pending):
                state, t_ = pending.pop(0)
                emit_mm2(state, t_)
                if t_ == KT_TILES - 1:
                    evacuate(state["bh"], state["out_ps"])

            pending_mm2 = []
            for bh in range(BH):
                zc0 = small_pool.tile([P, KT_TILES], F32, tag="zc0")
                zc1 = small_pool.tile([P, KT_TILES], F32, tag="zc1")
                nc.vector.memset(zc1[:], 0.0)
                ets = {}
                vp = small_pool.tile([P, KT_TILES, D], F16, tag="vp")
                out_ps = o_pool.tile([2 * D, T // 2], F32, tag="out_ps")
                head_state = {"bh": bh, "out_ps": out_ps, "ets": ets, "vp": vp}

                def emit_norm(bh_, t0, n, zc0=zc0, zc1=zc1, vp=vp):
                    ztot = small_pool.tile([P, n], F32, tag="ztot")
                    nc.vector.tensor_add(
                        ztot[:], zc0[:, t0:t0 + n], zc1[:, t0:t0 + n]
                    )
                    rec = small_pool.tile([P, n], F32, tag="rec")
                    nc.vector.reciprocal(rec[:], ztot[:])
                    nc.vector.scalar_tensor_tensor(
                        vp[:, t0:t0 + n, :], v_sb[:, bh_, t0:t0 + n, :], 1.0,
                        rec[:, :, None].broadcast_to([P, n, D]),
                        mybir.AluOpType.mult, mybir.AluOpType.mult,
                    )
                dve_tiles = dve_tiles_for_head(bh, bh == BH - 1)
                n_deep = 0  # 3-level gpsimd trees used so far this head
                for j in range(KT_TILES // 2):
                    tA, tB = 2 * j, 2 * j + 1
                    ets[tA] = et_pool.tile([P, T], I16, tag="et", name="etA")
                    ets[tB] = et_pool.tile([P, T], I16, tag="et", name="etB")
                    lhsA = kt_sb[0:D, bh, j * P:(j + 1) * P]
                    lhsB = kt_sb[D:2 * D, bh, j * P:(j + 1) * P]
                    for q0 in range(0, T, 1024):
                        spA = s_pool.tile([P, 1024], F32, tag="sp", name="spA")
                        spB = s_pool.tile([P, 1024], F32, tag="sp", name="spB")
                        for c in range(0, 1024, 512):
                            nc.tensor.matmul(
                                spA[:, c:c + 512],
                                lhsT=lhsA,
                                rhs=qt_sb[0:D, bh, q0 + c:q0 + c + 512],
                                start=True,
                                stop=True,
                            )
                            nc.tensor.matmul(
                                spB[:, c:c + 512],
                                lhsT=lhsB,
                                rhs=qt_sb[D:2 * D, bh, q0 + c:q0 + c + 512],
                                start=True,
                                stop=True,
                            )
                        for t, sp in ((tA, spA), (tB, spB)):
                            if t in dve_tiles:
                                nc.vector.tensor_scalar(
                                    ets[t][:, q0:q0 + 1024], sp[:],
                                    A_IMM, B_IMM,
                                    mybir.AluOpType.mult,
                                    mybir.AluOpType.add,
                                )
                            else:
                                zc = zc0 if q0 == 0 else zc1
                                nc.scalar.activation(
                                    ets[t].bitcast(F16)[:, q0:q0 + 1024],
                                    sp[:],
                                    mybir.ActivationFunctionType.Exp,
                                    scale=SCALE,
                                    accum_out=zc[:, t:t + 1],
                                )
                    # Sum path for DVE tiles: gpsimd fp16 add-tree (depth
                    # balanced against gpsimd saturation), small DVE reduce.
                    for t in (tA, tB):
                        if t not in dve_tiles:
                            continue
                        etf = ets[t].bitcast(F16)
                        h1 = ph_pool.tile([P, T // 2], F16, tag="h1")
                        h2 = ph_pool.tile([P, T // 4], F16, tag="h2")
                        nc.gpsimd.tensor_add(
                            h1[:], etf[:, 0:T // 2], etf[:, T // 2:T]
                        )
                        nc.gpsimd.tensor_add(
                            h2[:], h1[:, 0:T // 4], h1[:, T // 4:T // 2]
                        )
                        if n_deep < N_DEEP_LIMIT:
                            n_deep += 1
                            h3 = ph_pool.tile([P, T // 8], F16, tag="h3")
                            nc.gpsimd.tensor_add(
                                h3[:], h2[:, 0:T // 8], h2[:, T // 8:T // 4]
                            )
                            red_src = h3
                        else:
                            red_src = h2
                        nc.vector.tensor_reduce(
                            zc0[:, t:t + 1], red_src[:],
                            mybir.AxisListType.X, mybir.AluOpType.add,
                        )
                    # Last head: per-pair Z/vp so its mm2s pipeline instead
                    # of piling into a dead tail after all exps finish.
                    if bh == BH - 1:
                        emit_norm(bh, 2 * j, 2)
                        pending_mm2.append((head_state, 2 * j))
                        pending_mm2.append((head_state, 2 * j + 1))
                        # Lag pops one pair behind the norm emission so the
                        # PE never head-of-line blocks on an unready vp.
                        while len(pending_mm2) > 2:
                            pop_mm2(pending_mm2)
                    else:
                        # Drain the previous head's mm2s (vp long ready).
                        while len(pending_mm2) > 4:
                            pop_mm2(pending_mm2)

                if bh < BH - 1:
                    # Batched per-head normalization.
                    emit_norm(bh, 0, KT_TILES)
                    for t in range(KT_TILES):
                        pending_mm2.append((head_state, t))
            while pending_mm2:
                pop_mm2(pending_mm2)

    nc.compile()
    return nc


_NC_CACHE: dict = {}

TRACE = False
LAST_RESULTS = None


def _get_nc(BH: int, T: int):
    key = (BH, T)
    if key not in _NC_CACHE:
        _NC_CACHE[key] = build_attention_nc(BH, T)
    return _NC_CACHE[key]


def _reference_numpy(Q, K, V, padding_mask, isCausal):
    """Fallback exactly mirroring reference.py (never hit for spec inputs)."""
    Q = Q.astype(np.float64)
    K = K.astype(np.float64)
    V = V.astype(np.float64)
    scores = np.einsum("bhqd,bhkd->bhqk", Q, K) * SCALE
    T1 = scores.shape[2]
    mask = padding_mask[:, None, :, :].astype(np.float64)
    if isCausal:
        mask = mask * np.tril(np.ones((T1, T1)))
    scores = np.where(mask == 0, -np.inf, scores)
    m = np.max(scores, axis=2, keepdims=True)
    e = np.exp(scores - m)
    attn = e / np.sum(e, axis=2, keepdims=True)
    return np.einsum("bhqk,bhkd->bhqd", attn, V).astype(np.float32)


def kernel(Q, K, V, padding_mask, isCausal, **_unused):
    Q = np.asarray(Q, dtype=np.float32)
    K = np.asarray(K, dtype=np.float32)
    V = np.asarray(V, dtype=np.float32)
    padding_mask = np.asarray(padding_mask)
    causal = int(np.asarray(isCausal))

    B, H, T, Dd = Q.shape
    assert Dd == D
    if causal != 0 or padding_mask.min() != 1.0 or padding_mask.max() != 1.0:
        return _reference_numpy(Q, K, V, padding_mask, causal)

    BHT = B * H
    assert BHT % N_CORES == 0
    BH = BHT // N_CORES

    nc = _get_nc(BH, T)

    Qf = Q.reshape(BHT, T, D)
    Kf = K.reshape(BHT, T, D)
    Vf = V.reshape(BHT, T, D)

    QT = Qf.transpose(0, 2, 1).astype(np.float16)
    qt_all = np.ascontiguousarray(np.concatenate([QT, QT], axis=1))
    KT = Kf.transpose(0, 2, 1).astype(np.float16)
    KT4 = KT.reshape(BHT, D, T // 128, 128)
    kt_all = np.ascontiguousarray(
        np.concatenate(
            [
                KT4[:, :, 0::2, :].reshape(BHT, D, T // 2),
                KT4[:, :, 1::2, :].reshape(BHT, D, T // 2),
            ],
            axis=1,
        )
    )
    v_all = np.ascontiguousarray(
        Vf.reshape(BHT, T // P, P, D).transpose(0, 2, 1, 3)
    )

    in_maps = []
    for c in range(N_CORES):
        sl = slice(c * BH, (c + 1) * BH)
        in_maps.append(
            {
                "qt": np.ascontiguousarray(qt_all[sl]),
                "kt": np.ascontiguousarray(kt_all[sl]),
                "v": np.ascontiguousarray(v_all[sl]),
            }
        )

    res = None
    last_err = None
    for attempt in range(3):
        try:
            res = run_bass_kernel_spmd(
                nc, in_maps, core_ids=list(range(N_CORES)), trace=TRACE
            )
            break
        except Exception as e:
            last_err = e
            import time as _time

            _time.sleep(2.0)
    if res is None:
        raise last_err
    global LAST_RESULTS
    LAST_RESULTS = res

    outs = [res.results[c]["out"] for c in range(N_CORES)]
    out_all = np.concatenate(outs, axis=0)
    out = out_all.transpose(0, 2, 1).reshape(B, H, T, D)
    return np.ascontiguousarray(out).astype(np.float32)
